# revision 34
# baseline (speedup 1.0000x reference)
"""DeltaNet forward on 8 Trainium2 NeuronCores.

Sharding: core c -> batch b = c//2, head-pair hp = c%2 (heads 2hp, 2hp+1).
Each core computes its (b, head-pair) slice end-to-end in d-major layout and
produces a partial output projection; host sums the two partials per batch.

Math notes (validated in numpy against the reference):
 - (I+N)^-1 for the strictly-lower chunk matrix N via truncated Neumann
   product (I-N)(I+N^2)(I+N^4): exact to ~1e-5 for this data regime.
 - q/k l2-normalization via rsqrt(ones-matmul of squares) + PE row broadcast.
 - rms_w is folded into Wo on the host; D^-0.5 folded into q's rsqrt.
 - State tracked as Sk = S^T; masks pre-negated to avoid extra negations.
"""
import os
import sys
import types
import numpy as np
from contextlib import ExitStack

import concourse.bass as bass
import concourse.tile as tile
from concourse import mybir, bacc
from concourse.bass_utils import run_bass_kernel_spmd

F32 = mybir.dt.float32
F32R = mybir.dt.float32r

HID, H, D = 1024, 4, 256
KW, C, EPS = 4, 64, 1e-5
L = 4096
CB = 512                  # L-block
NB = L // CB              # 8 blocks
NCPB = CB // C            # 8 chunks per block
AOP = mybir.AluOpType
KPHASE = int(os.environ.get("KPHASE", "3"))


def build_core_kernel():
    nc = bacc.Bacc("TRN2", target_bir_lowering=False, debug=False, num_devices=8)

    xT = nc.dram_tensor("xT", [HID, L], F32R, kind="ExternalInput").ap()
    Wq = nc.dram_tensor("Wq", [HID, 512], F32R, kind="ExternalInput").ap()
    Wk = nc.dram_tensor("Wk", [HID, 512], F32R, kind="ExternalInput").ap()
    Wv = nc.dram_tensor("Wv", [HID, 512], F32R, kind="ExternalInput").ap()
    Wb = nc.dram_tensor("Wb", [HID, 128], F32R, kind="ExternalInput").ap()
    Wo = nc.dram_tensor("Wo", [512, HID], F32R, kind="ExternalInput").ap()
    # conv taps: (128, combo(4), tensor(3: q,k,v), tap(4))
    cw = nc.dram_tensor("cw", [128, 4, 3, KW], F32, kind="ExternalInput").ap()
    # consts: [I128 | stril(-1) | striu(+1) | -triu(0) | I64] = (128, 384)
    consts = nc.dram_tensor("consts", [128, 384], F32R, kind="ExternalInput").ap()

    out = nc.dram_tensor("out", [L, HID], F32, kind="ExternalOutput").ap()
    SkO = nc.dram_tensor("SkO", [128, 2, 2, D], F32R, kind="ExternalOutput").ap()

    with tile.TileContext(nc) as tc, ExitStack() as ctx, \
            nc.allow_low_precision(reason="float32r tiles are 4-byte fp32-width"):
        pool1 = ctx.enter_context(tc.tile_pool(name="p1", bufs=1))
        poolD = ctx.enter_context(tc.tile_pool(name="pD", bufs=2, space="DRAM"))
        poolW = ctx.enter_context(tc.tile_pool(name="pW", bufs=1))
        poolX = ctx.enter_context(tc.tile_pool(name="pX", bufs=1))
        poolQ = ctx.enter_context(tc.tile_pool(name="pQ", bufs=1))
        poolA = ctx.enter_context(tc.tile_pool(name="pA", bufs=1))
        poolR = ctx.enter_context(tc.tile_pool(name="pR", bufs=2))
        poolS = ctx.enter_context(tc.tile_pool(name="pS", bufs=1))
        poolO = ctx.enter_context(tc.tile_pool(name="pO", bufs=1))
        ppP = ctx.enter_context(tc.tile_pool(name="ppP", bufs=5, space="PSUM"))
        ppA = ctx.enter_context(tc.tile_pool(name="ppA", bufs=3, space="PSUM"))


        def midb(ap2d, n):
            # (P, F) -> (P, n, F) with step-0 middle dim
            return bass.AP(tensor=ap2d.tensor, offset=ap2d.offset,
                           ap=[ap2d.ap[0], [0, n], ap2d.ap[1]])

        def pp_big():
            return ppP.tile([128, 512], F32, tag="pp", name="pp")

        def pp_sm():
            return ppP.tile([128, 512], F32, tag="pp", name="pp")

        def pp_smr():
            return ppA.tile([64, 512], F32R, tag="pa", name="pa")

        def pp_bigr():
            return ppA.tile([128, 512], F32R, tag="pa", name="pa")

        # ---- constants / weights resident in SBUF ----
        csb = pool1.tile([128, 384], F32R, tag="consts")
        nc.sync.dma_start(out=csb, in_=consts)
        I128 = csb[:, 0:128]
        stril = csb[0:64, 128:192]        # strict lower ones
        striu = csb[0:64, 192:256]        # strict upper ones
        ntri = csb[0:64, 256:320]         # -(upper incl diag)
        I64 = csb[0:64, 320:384]

        cwsb = pool1.tile([128, 4, 3, KW], F32, tag="cw")
        nc.sync.dma_start(out=cwsb, in_=cw)

        ones_col = pool1.tile([128, 128], F32R, tag="ones_col")
        nc.vector.memset(ones_col.bitcast(F32), 0.0)
        nc.vector.memset(ones_col.bitcast(F32)[:, 0:1], 1.0)
        eps_sb = pool1.tile([128, 1], F32, tag="eps_sb")
        nc.vector.memset(eps_sb, EPS)
        ones_row = pool1.tile([128, 128], F32R, tag="ones_row")
        nc.vector.memset(ones_row.bitcast(F32), 1.0)

        Wsb = {}
        for name, t in (("q", Wq), ("k", Wk), ("v", Wv)):
            w = poolW.tile([128, 8, 512], F32R, tag=f"W{name}")
            nc.sync.dma_start(out=w, in_=t.rearrange("(o p) n -> p o n", p=128))
            Wsb[name] = w
        Wbsb = poolW.tile([128, 8, 128], F32R, tag="Wb")
        nc.sync.dma_start(out=Wbsb, in_=Wb.rearrange("(o p) n -> p o n", p=128))
        Wosb = poolW.tile([128, 4, HID], F32R, tag="Wo")
        nc.sync.dma_start(out=Wosb, in_=Wo.rearrange("(o p) n -> p o n", p=128))

        # persistent state
        Sk = poolQ.tile([128, 2, 2, D], F32R, tag="Sk")     # [p, head, dkt, dv]
        nc.vector.memset(Sk.bitcast(F32), 0.0)
        carry = poolQ.tile([128, 4, 3, 3], F32, tag="carry")  # [p, combo, tensor, tap]
        nc.vector.memset(carry, 0.0)

        for bi in range(NB):
            l0 = bi * CB
            xblk = poolX.tile([128, 8, CB], F32R, tag="xblk")
            for kk in range(8):
                nc.sync.dma_start(out=xblk[:, kk, :],
                                  in_=xT[kk * 128:(kk + 1) * 128, l0:l0 + CB])

            # ---- projections + conv + silu (d-major) ----
            blk = {}
            raws = {}
            for ti, name in enumerate(("q", "k", "v")):
                tb = poolA.tile([128, 4, CB], F32R, tag=f"{name}T")
                blk[name] = tb
                raw = poolR.tile([128, 4, CB + 3], F32, tag="raw", name=f"raw{name}")
                raws[name] = raw
                # restore carry (prev block's last 3 raw cols)
                nc.gpsimd.tensor_copy(out=raw[:, :, 0:3], in_=carry[:, :, ti, :])
                for combo in range(4):
                    ps = pp_big()
                    for kk in range(8):
                        nc.tensor.matmul(
                            ps, Wsb[name][:, kk, combo * 128:(combo + 1) * 128],
                            xblk[:, kk, :], start=(kk == 0), stop=(kk == 7))
                    nc.vector.tensor_copy(out=raw[:, combo, 3:3 + CB], in_=ps)
                # save carry for next block
                nc.gpsimd.tensor_copy(out=carry[:, :, ti, :], in_=raw[:, :, CB:CB + 3])
                # conv taps + silu per combo
                for combo in range(4):
                    y = poolS.tile([128, CB], F32, tag="convy", bufs=1, name="convy")
                    eng = nc.vector
                    eng.tensor_scalar(out=y, in0=raw[:, combo, 0:CB],
                                      scalar1=cwsb[:, combo, ti, 0:1],
                                      scalar2=None, op0=AOP.mult)
                    for t in range(1, KW):
                        eng.scalar_tensor_tensor(
                            out=y, in0=raw[:, combo, t:t + CB],
                            scalar=cwsb[:, combo, ti, t:t + 1],
                            in1=y, op0=AOP.mult, op1=AOP.add)
                    nc.scalar.activation(out=tb[:, combo, :], in_=y,
                                         func=mybir.ActivationFunctionType.Silu)

            if KPHASE == 1:
                for combo in range(4):
                    nc.sync.dma_start(
                        out=out[l0 + combo * 128: l0 + (combo + 1) * 128, 0:CB],
                        in_=blk["q"][:, combo, :].bitcast(F32))
                continue
            # ---- beta (row form (2, CB)) ----
            psb = pp_big()
            for kk in range(8):
                nc.tensor.matmul(psb, Wbsb[:, kk, :], xblk[:, kk, :],
                                 start=(kk == 0), stop=(kk == 7))
            brow = poolA.tile([128, CB], F32R, tag="brow")
            for hh in range(2):
                nc.scalar.activation(out=brow[hh * 64:hh * 64 + 1, :],
                                     in_=psb[hh * 64:hh * 64 + 1, :],
                                     func=mybir.ActivationFunctionType.Sigmoid)
            # beta column form (64, chunk, head) via DRAM bounce (transpose AP)
            bcol = poolA.tile([64, 2, NCPB], F32R, tag="bcol")
            dscr = poolD.tile([2, CB], F32R, tag="dscr", name="dscr")
            nc.sync.dma_start(out=dscr, in_=brow[0:128:64, :])
            nc.sync.dma_start(out=bcol,
                              in_=dscr.rearrange("h (ch c) -> c h ch", c=64))

            # ---- q/k normalization ----
            rrow = {}
            for ti, name in (("q", "q"), ("k", "k")):
                tb = blk[name]
                sq = poolR.tile([128, 4, CB], F32R, tag="raw", name="sq")
                nc.gpsimd.tensor_mul(sq, tb, tb)
                rr = poolA.tile([128, CB], F32R, tag=f"rr{name}")
                for hh in range(2):
                    psr = pp_big()
                    for dt in range(2):
                        nc.tensor.matmul(psr, ones_col,
                                         sq[:, hh * 2 + dt, :],
                                         start=(dt == 0), stop=(dt == 1))
                    nc.scalar.activation(out=rr[hh * 64:hh * 64 + 1, :],
                                         in_=psr[0:1, 0:CB],
                                         func=mybir.ActivationFunctionType.Sqrt,
                                         scale=(float(D) if name == "q" else 1.0))
                    nc.vector.reciprocal(out=rr[hh * 64:hh * 64 + 1, :],
                                         in_=rr[hh * 64:hh * 64 + 1, :])
                rrow[name] = rr
                for hh in range(2):
                    psn = pp_big()
                    nc.tensor.matmul(psn, ones_row[hh * 64:hh * 64 + 1, 0:128], rr[hh * 64:hh * 64 + 1, :],
                                     start=True, stop=True)
                    nc.vector.tensor_tensor(
                        out=tb[:, hh * 2:hh * 2 + 2, :],
                        in0=tb[:, hh * 2:hh * 2 + 2, :],
                        in1=midb(psn, 2),
                        op=AOP.mult)

            if KPHASE == 2:
                for combo in range(4):
                    nc.sync.dma_start(
                        out=out[l0 + combo * 128: l0 + (combo + 1) * 128, 0:CB],
                        in_=blk["q"][:, combo, :].bitcast(F32))
                continue
            # ---- stage A per head ----
            TTs, ATs, KLs, VLs = [], [], [], []
            for hh in range(2):
                kTh = blk["k"][:, hh * 2:hh * 2 + 2, :]
                qTh = blk["q"][:, hh * 2:hh * 2 + 2, :]
                vTh = blk["v"][:, hh * 2:hh * 2 + 2, :]
                bc_h = bcol[:, hh, :]                      # (64, 8)

                # G (64, chunk, 64): lhsT padded to M=128 (fp32r needs col_grp=0xf);
                # chunk cI result lands in psum rows [RO(cI):RO(cI)+64]
                def LO(cI):
                    return cI * 64 if cI < 7 else 6 * 64

                def RO(cI):
                    return 0 if cI < 7 else 64

                psG = pp_sm()
                for cI in range(NCPB):
                    for dt in range(2):
                        nc.tensor.matmul(psG[:, cI * 64:(cI + 1) * 64],
                                         kTh[:, dt, LO(cI):LO(cI) + 128],
                                         kTh[:, dt, cI * 64:(cI + 1) * 64],
                                         start=(dt == 0), stop=(dt == 1))
                Gsb = poolS.tile([64, NCPB, 64], F32R, tag="Gsb", name=f"Gsb{hh}")
                psGv = psG.rearrange("p (c n) -> p c n", c=NCPB)
                nc.scalar.copy(out=Gsb[:, 0:7, :], in_=psGv[0:64, 0:7, :])
                nc.scalar.copy(out=Gsb[:, 7, :], in_=psGv[64:128, 7, :])
                # beta row-broadcast (64, CB) psum for column scaling
                psbb = pp_sm()
                nc.tensor.matmul(psbb[:, 0:CB], ones_row[hh * 64:hh * 64 + 1, 0:128],
                                 brow[hh * 64:hh * 64 + 1, :], start=True, stop=True)
                # N = stril . (bcol * G);  M = striu . (G * b_bcast)
                Nt = poolS.tile([64, NCPB, 64], F32R, tag="Nt", name="Nt")
                Mt = poolS.tile([64, NCPB, 64], F32R, tag="Mt", name="Mt")
                nc.vector.tensor_tensor(out=Nt, in0=Gsb,
                                        in1=bc_h.to_broadcast([64, NCPB, 64]),
                                        op=AOP.mult)
                nc.vector.tensor_tensor(out=Nt, in0=Nt,
                                        in1=midb(stril, NCPB),
                                        op=AOP.mult)
                nc.vector.tensor_tensor(out=Mt, in0=Gsb,
                                        in1=psbb[0:64, 0:CB].rearrange("p (c n) -> p c n", c=NCPB),
                                        op=AOP.mult)
                nc.vector.tensor_tensor(out=Mt, in0=Mt,
                                        in1=midb(striu, NCPB),
                                        op=AOP.mult)
                # T-chain: N2, M2 -> N4; P0 = I64 - M; P1 = P0 + M2@P0; TT = P1 + M4@P1
                Ntf = Nt.rearrange("p c n -> p (c n)")
                Mtf = Mt.rearrange("p c n -> p (c n)")
                psN2 = pp_sm()
                psM2 = pp_sm()
                for cI in range(NCPB):
                    sl = slice(cI * 64, (cI + 1) * 64)
                    nc.tensor.matmul(psN2[:, sl], Mtf[:, LO(cI):LO(cI) + 128],
                                     Nt[:, cI, :], start=True, stop=True)
                    nc.tensor.matmul(psM2[:, sl], Ntf[:, LO(cI):LO(cI) + 128],
                                     Mt[:, cI, :], start=True, stop=True)
                N2 = poolS.tile([64, NCPB, 64], F32R, tag="N2", name="N2")
                M2 = poolS.tile([64, NCPB, 64], F32R, tag="M2", name="M2")
                for (dst, ps) in ((N2, psN2), (M2, psM2)):
                    v = ps.rearrange("p (c n) -> p c n", c=NCPB)
                    nc.scalar.copy(out=dst[:, 0:7, :], in_=v[0:64, 0:7, :])
                    nc.scalar.copy(out=dst[:, 7, :], in_=v[64:128, 7, :])
                M2f = M2.rearrange("p c n -> p (c n)")
                psN4 = pp_sm()
                for cI in range(NCPB):
                    nc.tensor.matmul(psN4[:, cI * 64:(cI + 1) * 64],
                                     M2f[:, LO(cI):LO(cI) + 128],
                                     N2[:, cI, :], start=True, stop=True)
                N4 = poolS.tile([64, NCPB, 64], F32R, tag="N4", name="N4")
                v = psN4.rearrange("p (c n) -> p c n", c=NCPB)
                nc.scalar.copy(out=N4[:, 0:7, :], in_=v[0:64, 0:7, :])
                nc.scalar.copy(out=N4[:, 7, :], in_=v[64:128, 7, :])
                P0 = poolS.tile([64, NCPB, 64], F32R, tag="P0", name="P0")
                nc.vector.tensor_tensor(out=P0,
                                        in0=midb(I64, NCPB),
                                        in1=Mt, op=AOP.subtract)
                N2f = N2.rearrange("p c n -> p (c n)")
                psP = pp_sm()
                for cI in range(NCPB):
                    nc.tensor.matmul(psP[:, cI * 64:(cI + 1) * 64],
                                     N2f[:, LO(cI):LO(cI) + 128],
                                     P0[:, cI, :], start=True, stop=True)
                P1 = poolS.tile([64, NCPB, 64], F32R, tag="P1", name="P1")
                v = psP.rearrange("p (c n) -> p c n", c=NCPB)
                nc.vector.tensor_tensor(out=P1[:, 0:7, :], in0=P0[:, 0:7, :],
                                        in1=v[0:64, 0:7, :], op=AOP.add)
                nc.vector.tensor_tensor(out=P1[:, 7, :], in0=P0[:, 7, :],
                                        in1=v[64:128, 7, :], op=AOP.add)
                N4f = N4.rearrange("p c n -> p (c n)")
                psT = pp_sm()
                for cI in range(NCPB):
                    nc.tensor.matmul(psT[:, cI * 64:(cI + 1) * 64],
                                     N4f[:, LO(cI):LO(cI) + 128],
                                     P1[:, cI, :], start=True, stop=True)
                TT = poolS.tile([64, NCPB, 64], F32R, tag=f"TT{hh}", name=f"TT{hh}")
                v = psT.rearrange("p (c n) -> p c n", c=NCPB)
                nc.vector.tensor_tensor(out=TT[:, 0:7, :], in0=P1[:, 0:7, :],
                                        in1=v[0:64, 0:7, :], op=AOP.add)
                nc.vector.tensor_tensor(out=TT[:, 7, :], in0=P1[:, 7, :],
                                        in1=v[64:128, 7, :], op=AOP.add)
                TTs.append(TT)

                # AttnTn = -(triu incl) . (K^ Q^T)
                psKQ = pp_sm()
                for cI in range(NCPB):
                    for dt in range(2):
                        nc.tensor.matmul(psKQ[:, cI * 64:(cI + 1) * 64],
                                         kTh[:, dt, LO(cI):LO(cI) + 128],
                                         qTh[:, dt, cI * 64:(cI + 1) * 64],
                                         start=(dt == 0), stop=(dt == 1))
                AT = poolS.tile([64, NCPB, 64], F32R, tag=f"AT{hh}", name=f"AT{hh}")
                v = psKQ.rearrange("p (c n) -> p c n", c=NCPB)
                nc.vector.tensor_tensor(out=AT[:, 0:7, :], in0=v[0:64, 0:7, :],
                                        in1=midb(ntri, 7), op=AOP.mult)
                nc.vector.tensor_tensor(out=AT[:, 7, :], in0=v[64:128, 7, :],
                                        in1=ntri, op=AOP.mult)
                ATs.append(AT)

                # k^ and beta*v transposed to L-major via PE transpose
                KL = poolS.tile([64, NCPB, D], F32R, tag=f"KL{hh}", name=f"KL{hh}")
                VL = poolS.tile([64, NCPB, D], F32R, tag=f"VL{hh}", name=f"VL{hh}")
                for pair in range(NCPB // 2):
                    psK = pp_smr()
                    psV = pp_smr()
                    for j in range(2):
                        cI = pair * 2 + j
                        for dt in range(2):
                            nc.tensor.transpose(
                                psK[:, j * 256 + dt * 128: j * 256 + (dt + 1) * 128],
                                kTh[:, dt, cI * 64:(cI + 1) * 64], I128)
                            nc.tensor.transpose(
                                psV[:, j * 256 + dt * 128: j * 256 + (dt + 1) * 128],
                                vTh[:, dt, cI * 64:(cI + 1) * 64], I128)
                    nc.scalar.copy(out=KL[:, pair * 2:pair * 2 + 2, :],
                                   in_=psK.rearrange("p (c n) -> p c n", c=2))
                    nc.vector.tensor_tensor(
                        out=VL[:, pair * 2:pair * 2 + 2, :],
                        in0=psV.rearrange("p (c n) -> p c n", c=2),
                        in1=bc_h[:, pair * 2:pair * 2 + 2].to_broadcast([64, 2, D]),
                        op=AOP.mult)
                KLs.append(KL)
                VLs.append(VL)

            if KPHASE == 25:
                for combo in range(4):
                    nc.sync.dma_start(
                        out=out[l0 + combo * 128: l0 + (combo + 1) * 128, 0:CB],
                        in_=blk["q"][:, combo, :].bitcast(F32))
                continue
            # ---- stage B: sequential chunks ----
            Oblk = poolR.tile([128, 2, 4, D], F32R, tag="raw", name="Oblk")
            def LO(cI):
                return cI * 64 if cI < 7 else 6 * 64

            def RO(cI):
                return 0 if cI < 7 else 64

            for idx in range(NCPB):
                for hh in range(2):
                    kTh = blk["k"][:, hh * 2:hh * 2 + 2, :]
                    qTh = blk["q"][:, hh * 2:hh * 2 + 2, :]
                    lo, ro = LO(idx), RO(idx)
                    TTf = TTs[hh].rearrange("p c n -> p (c n)")
                    ATf = ATs[hh].rearrange("p c n -> p (c n)")
                    psKS = pp_sm()
                    for dt in range(2):
                        nc.tensor.matmul(psKS[:, 0:D], kTh[:, dt, lo:lo + 128],
                                         Sk[:, hh, dt, :], start=(dt == 0), stop=(dt == 1))
                    Em = poolS.tile([64, D], F32R, tag="Em", bufs=2, name="Em")
                    nc.vector.scalar_tensor_tensor(
                        out=Em, in0=psKS[ro:ro + 64, 0:D], scalar=bcol[:, hh, idx:idx + 1],
                        in1=VLs[hh][:, idx, :], op0=AOP.mult, op1=AOP.subtract)
                    psmid = pp_sm()
                    nc.tensor.matmul(psmid[:, 0:D], TTf[:, lo:lo + 128], Em,
                                     start=True, stop=True)
                    midn = poolS.tile([64, D], F32R, tag="midn", bufs=2, name="midn")
                    nc.scalar.copy(out=midn, in_=psmid[ro:ro + 64, 0:D])
                    psO = pp_sm()
                    for dt in range(2):
                        nc.tensor.matmul(psO[:, 0:D], qTh[:, dt, lo:lo + 128],
                                         Sk[:, hh, dt, :], start=(dt == 0), stop=False)
                    nc.tensor.matmul(psO[:, 0:D], ATf[:, lo:lo + 128], midn,
                                     start=False, stop=True)
                    nc.scalar.copy(
                        out=Oblk[(idx % 2) * 64:(idx % 2) * 64 + 64, hh, idx // 2, :],
                        in_=psO[ro:ro + 64, 0:D])
                    psdS = pp_big()
                    for dt in range(2):
                        nc.tensor.matmul(psdS[:, dt * D:(dt + 1) * D],
                                         KLs[hh][:, idx, dt * 128:(dt + 1) * 128],
                                         midn, start=True, stop=True)
                    nc.vector.tensor_tensor(out=Sk[:, hh], in0=Sk[:, hh],
                                            in1=psdS[:, 0:512].rearrange("p (t d) -> p t d", t=2),
                                            op=AOP.subtract)

            # ---- RMS norm + transpose + output projection ----
            oT = poolA.tile([128, 4, CB], F32R, tag="qT", name="oT")
            for hh in range(2):
                osq = poolR.tile([128, 4, D], F32, tag="raw", name="osq")
                nc.gpsimd.tensor_mul(osq, Oblk[:, hh], Oblk[:, hh])
                red = poolS.tile([128, 4, 1], F32, tag="red")
                nc.vector.tensor_reduce(out=red, in_=osq, axis=mybir.AxisListType.X,
                                        op=AOP.add)
                rms = poolS.tile([128, 4, 1], F32, tag="rms")
                nc.scalar.activation(out=rms, in_=red,
                                     func=mybir.ActivationFunctionType.Sqrt,
                                     bias=eps_sb, scale=1.0 / D)
                nc.vector.reciprocal(out=rms, in_=rms)
                nc.vector.tensor_tensor(out=Oblk[:, hh], in0=Oblk[:, hh],
                                        in1=rms.to_broadcast([128, 4, D]), op=AOP.mult)
                for dt in range(2):
                    psot = pp_bigr()
                    for lt in range(4):
                        nc.tensor.transpose(psot[:, lt * 128:(lt + 1) * 128],
                                            Oblk[:, hh, lt, dt * 128:(dt + 1) * 128],
                                            I128)
                    nc.scalar.copy(out=oT[:, hh * 2 + dt, :], in_=psot[:, 0:CB])
            for lt in range(4):
                for nn in range(2):
                    pso = pp_big()
                    for combo in range(4):
                        nc.tensor.matmul(pso, oT[:, combo, lt * 128:(lt + 1) * 128],
                                         Wosb[:, combo, nn * 512:(nn + 1) * 512],
                                         start=(combo == 0), stop=(combo == 3))
                    osb = poolS.tile([128, 512], F32, tag="osb", bufs=2, name="osb")
                    nc.scalar.copy(out=osb, in_=pso)
                    nc.sync.dma_start(
                        out=out[l0 + lt * 128: l0 + (lt + 1) * 128,
                                nn * 512:(nn + 1) * 512],
                        in_=osb)

        nc.sync.dma_start(out=SkO, in_=Sk)

    nc.compile()
    return nc


_NC_CACHE = {}


def _get_nc():
    if "nc" not in _NC_CACHE:
        _NC_CACHE["nc"] = build_core_kernel()
    return _NC_CACHE["nc"]


def _wb_pad(Wb, hp):
    w = np.zeros((HID, 128), np.float32)
    w[:, 0] = Wb[:, hp * 2]
    w[:, 64] = Wb[:, hp * 2 + 1]
    return w


def kernel(x, Wq, Wk, Wv, Wb, conv_q_w, conv_k_w, conv_v_w, rms_w, Wo):
    x = np.asarray(x, np.float32)
    B = x.shape[0]
    Wo_f = (np.asarray(Wo) * np.tile(np.asarray(rms_w), H)[:, None]).astype(np.float32)

    # constants
    consts = np.zeros((128, 384), np.float32)
    consts[:, 0:128] = np.eye(128, dtype=np.float32)
    consts[0:64, 128:192] = np.tril(np.ones((64, 64), np.float32), -1)
    consts[0:64, 192:256] = np.triu(np.ones((64, 64), np.float32), 1)
    consts[0:64, 256:320] = -np.triu(np.ones((64, 64), np.float32))
    consts[0:64, 320:384] = np.eye(64, dtype=np.float32)

    cws = {"q": np.asarray(conv_q_w), "k": np.asarray(conv_k_w),
           "v": np.asarray(conv_v_w)}

    in_maps = []
    for core in range(8):
        b, hp = core // 2, core % 2
        cols = slice(hp * 512, hp * 512 + 512)
        cw = np.zeros((128, 4, 3, KW), np.float32)
        for ti, name in enumerate(("q", "k", "v")):
            w = cws[name][cols, 0, :]         # (512, KW)
            cw[:, :, ti, :] = w.reshape(4, 128, KW).transpose(1, 0, 2)
        in_maps.append({
            "xT": np.ascontiguousarray(x[b].T),
            "Wq": np.ascontiguousarray(np.asarray(Wq)[:, cols]),
            "Wk": np.ascontiguousarray(np.asarray(Wk)[:, cols]),
            "Wv": np.ascontiguousarray(np.asarray(Wv)[:, cols]),
            "Wb": _wb_pad(np.asarray(Wb), hp),
            "Wo": np.ascontiguousarray(Wo_f[cols, :]),
            "cw": cw,
            "consts": consts,
        })

    nc = _get_nc()
    res = run_bass_kernel_spmd(nc, in_maps, core_ids=list(range(8)))
    kernel.last_results = res

    o = np.zeros((B, L, HID), np.float32)
    S = np.zeros((B, H, D, D), np.float32)
    for core in range(8):
        b, hp = core // 2, core % 2
        o[b] += res.results[core]["out"]
        sk = res.results[core]["SkO"]        # (128, 2, 2, D) = [p, h, t, d]
        for hh in range(2):
            S[b, hp * 2 + hh] = sk[:, hh].transpose(1, 0, 2).reshape(D, D).T
    return o, S
